# revision 8
# baseline (speedup 1.0000x reference)
"""Trainium2 Bass kernel for nn_AggregatorL1 (GNN message passing).

    self_out  = emb[x0[b]] @ W_self.T  + b_self
    neigh_out = mean_j(emb[x1[b, j]]) @ W_neigh.T + b_neigh
    out[b]    = relu(concat([self_out, neigh_out]))

Distribution: data-parallel over the batch across 8 NeuronCores (2048
nodes per core); embedding table and weights replicated.

Per-core dataflow (all-bf16 datapath; fp32 only in PSUM accumulation):
  * One unified draw stream per 32768-row vocab window (int16 gather
    index range): for each (window, 128-node block) segment, the block's
    SELF draws come first (they fit slot 0), then its neighbor draws,
    padded to 128-position slots. SWDGE `dma_gather` fetches rows from a
    bf16 table copy in a few large gathers (GG blocks per gather, spread
    over 4 SWDGE queues) - position i lands at [i%128, i//128].
  * Per block, rows are reduced to per-node sums with TensorE matmuls in
    FEATURE-major orientation: psum[f_half, node] += G_slot[:, f_half].T
    @ A_slot, where A[p, n] = (tagA[p] == n) is built on-device by DVE
    is_equal from host tags (pad/self positions carry tag 255 -> zero).
    Slot 0 additionally accumulates the self sum via tagB columns.
  * Projection without transposes: out_psum[node, h] += SUM_fm[f_half,
    node].T @ W.T-chunk (weights bf16; the 1/32 neighbor mean is folded
    into W_neigh.T host-side), plus one bias matmul (lhsT = E with row 0
    of ones, rhs = bias broadcast row), then ScalarE relu PSUM -> SBUF
    and one [128, 512] DMA out per block.
"""

import os
import sys

sys.path.insert(0, "/opt/trn_rl_repo")

from contextlib import ExitStack

import ml_dtypes
import numpy as np

import concourse.bacc as bacc
import concourse.bass as bass
import concourse.mybir as mybir
import concourse.tile as tile
from concourse import library_config
from concourse.bass_utils import run_bass_kernel_spmd

N_CORES = 8
B = 16384
NNEIGH = 32
F = 256
H = 256
V = 100000
BPC = B // N_CORES  # 2048 nodes per core
NBLK = BPC // 128  # 16 blocks of 128 nodes
NQ = 4
CW = 32768  # vocab window width (int16 gather index range)
F32 = mybir.dt.float32
BF16 = mybir.dt.bfloat16
I16 = mybir.dt.int16

GG = int(os.environ.get("KGG", "2"))  # blocks per gather group
KQMODE = os.environ.get("KQMODE", "rr")  # gather queue pick: rr | win

_BUILT = {}


def _wrap16(stream_idx):
    """dma_gather index layout: wrapped[p, j] = stream[16*j + p]."""
    return np.ascontiguousarray(stream_idx.reshape(-1, 16).T)


def _host_prep(x0, x1):
    """Build the per-window draw streams (self-first segments per (q, blk)),
    with a core-independent slot structure so one SPMD program serves all
    cores; emit int16 index streams and bf16 tag matrices per core."""
    x0 = np.asarray(x0, dtype=np.int64)
    x1 = np.asarray(x1, dtype=np.int64)

    per_core = []
    for c in range(N_CORES):
        sl = slice(c * BPC, (c + 1) * BPC)
        per_core.append(
            {
                "vs": x0[sl],  # [BPC]
                "vn": x1[sl].reshape(-1),  # [BPC*NNEIGH], node-major
                "nn": np.repeat(np.arange(BPC), NNEIGH),
            }
        )

    # per (core, q, blk): self + neighbor counts
    sizes = np.zeros((N_CORES, NQ, NBLK), np.int64)
    sizes_s = np.zeros((N_CORES, NQ, NBLK), np.int64)
    for c in range(N_CORES):
        pc = per_core[c]
        np.add.at(sizes, (c, pc["vn"] >> 15, pc["nn"] >> 7), 1)
        np.add.at(sizes_s, (c, pc["vs"] >> 15, np.arange(BPC) >> 7), 1)
    sizes += sizes_s
    assert (sizes_s <= 128).all(), "self entries must fit in slot 0"
    slots = np.maximum(1, -(-sizes.max(axis=0) // 128))  # (q, blk)
    seg_start = np.zeros((NQ, NBLK + 1), np.int64)
    for q in range(NQ):
        seg_start[q, 1:] = np.cumsum(slots[q])

    CA = int(slots.sum())  # A columns: one per slot
    CB = NQ * NBLK  # B columns: slot 0 of each (q, blk)

    structure = {"slots": slots, "seg_start": seg_start, "CA": CA, "CB": CB}

    per_core_arrays = []
    for c in range(N_CORES):
        pc = per_core[c]
        arrs = {"idx": []}
        tagsA_streams = []
        tagsB_streams = []
        for q in range(NQ):
            L = int(seg_start[q][-1]) * 128
            stream_idx = np.zeros(L, np.int16)
            tA = np.full(L, 255.0, np.float32)
            tB = np.full(L, 255.0, np.float32)
            # self draws first within each (q, blk) segment
            sel_s = np.where((pc["vs"] >> 15) == q)[0]  # node ids
            bs_s = sel_s >> 7
            seg_first = np.searchsorted(bs_s, np.arange(NBLK), side="left")
            rank = np.arange(sel_s.size) - seg_first[bs_s]
            dest_s = 128 * seg_start[q][bs_s] + rank
            stream_idx[dest_s] = (pc["vs"][sel_s] - CW * q).astype(np.int16)
            tB[dest_s] = (sel_s & 127).astype(np.float32)
            n_self_blk = np.zeros(NBLK, np.int64)
            np.add.at(n_self_blk, bs_s, 1)
            # neighbor draws after the block's self draws
            sel_n = np.where((pc["vn"] >> 15) == q)[0]
            bs_n = pc["nn"][sel_n] >> 7  # node-major -> sorted by block
            seg_first = np.searchsorted(bs_n, np.arange(NBLK), side="left")
            rank = np.arange(sel_n.size) - seg_first[bs_n]
            dest_n = 128 * seg_start[q][bs_n] + n_self_blk[bs_n] + rank
            assert (rank + n_self_blk[bs_n] < 128 * slots[q][bs_n]).all()
            stream_idx[dest_n] = (pc["vn"][sel_n] - CW * q).astype(np.int16)
            tA[dest_n] = (pc["nn"][sel_n] & 127).astype(np.float32)

            w16 = _wrap16(stream_idx)
            groups = []
            for r in range(-(-NBLK // GG)):
                c0 = 8 * int(seg_start[q][GG * r])
                c1 = 8 * int(seg_start[q][min(NBLK, GG * (r + 1))])
                groups.append(np.ascontiguousarray(np.tile(w16[:, c0:c1], (8, 1))))
            arrs["idx"].append(groups)
            tagsA_streams.append(tA)
            tagsB_streams.append(tB)

        # A columns in consumption order: for blk: for q: for s
        tagsA = np.empty((128, CA), np.float32)
        tagsB = np.empty((128, CB), np.float32)
        ca = cb = 0
        for blk in range(NBLK):
            for q in range(NQ):
                base = 128 * seg_start[q][blk]
                for s in range(int(slots[q][blk])):
                    tagsA[:, ca] = tagsA_streams[q][base + 128 * s : base + 128 * (s + 1)]
                    ca += 1
                tagsB[:, cb] = tagsB_streams[q][base : base + 128]
                cb += 1
        assert ca == CA and cb == CB
        arrs["tagsA"] = np.ascontiguousarray(tagsA.astype(ml_dtypes.bfloat16))
        arrs["tagsB"] = np.ascontiguousarray(tagsB.astype(ml_dtypes.bfloat16))
        per_core_arrays.append(arrs)

    return structure, per_core_arrays


def _build(structure):
    slots = structure["slots"]
    seg_start = structure["seg_start"]
    CA, CB = structure["CA"], structure["CB"]
    NR = -(-NBLK // GG)  # gather groups per window
    ATILES = -(-CA // 16)
    BTILES = -(-CB // 16)

    nc = bacc.Bacc(None, target_bir_lowering=False, debug=True, num_swdge_queues=4)

    emb16 = nc.dram_tensor("emb16", [V, F], BF16, kind="ExternalInput")
    wst = nc.dram_tensor("wst", [F, H], BF16, kind="ExternalInput")  # W_self.T
    wnt = nc.dram_tensor("wnt", [F, H], BF16, kind="ExternalInput")  # W_neigh.T/32
    brow_d = nc.dram_tensor("brow", [128, 2 * H], BF16, kind="ExternalInput")
    eone_d = nc.dram_tensor("eone", [128, 128], BF16, kind="ExternalInput")
    iota16_d = nc.dram_tensor("iota16", [128, 16 * 128], BF16, kind="ExternalInput")
    tagsA_d = nc.dram_tensor("tagsA", [128, CA], BF16, kind="ExternalInput")
    tagsB_d = nc.dram_tensor("tagsB", [128, CB], BF16, kind="ExternalInput")
    idx_d = [
        [
            nc.dram_tensor(
                f"idx{q}_{r}",
                [
                    128,
                    8
                    * int(
                        seg_start[q][min(NBLK, GG * (r + 1))] - seg_start[q][GG * r]
                    ),
                ],
                I16,
                kind="ExternalInput",
            )
            for r in range(NR)
        ]
        for q in range(NQ)
    ]
    out = nc.dram_tensor("out", [BPC, 2 * H], F32, kind="ExternalOutput")

    SMAX = [
        max(
            int(seg_start[q][min(NBLK, GG * (r + 1))] - seg_start[q][GG * r])
            for r in range(NR)
        )
        for q in range(NQ)
    ]

    with tile.TileContext(nc) as tc, ExitStack() as ctx:
        const = ctx.enter_context(tc.tile_pool(name="const", bufs=1))
        GBUFS = int(os.environ.get("KGBUFS", "3"))
        gpools = [
            ctx.enter_context(tc.tile_pool(name=f"g{q}", bufs=GBUFS))
            for q in range(NQ)
        ]
        apool = ctx.enter_context(tc.tile_pool(name="a", bufs=6))
        bpool = ctx.enter_context(tc.tile_pool(name="b", bufs=2))
        sumpool = ctx.enter_context(tc.tile_pool(name="sum", bufs=4))
        opool = ctx.enter_context(tc.tile_pool(name="ostage", bufs=2))
        ps_sel = ctx.enter_context(tc.tile_pool(name="ps_sel", bufs=2, space="PSUM"))
        ps_out = ctx.enter_context(tc.tile_pool(name="ps_out", bufs=2, space="PSUM"))

        nc.gpsimd.load_library(library_config.mlp)

        wt = {}
        for path, dram in (("s", wst), ("n", wnt)):
            for k in range(2):
                t = const.tile([128, H], BF16, tag=f"w{path}{k}")
                nc.sync.dma_start(out=t[:], in_=dram[128 * k : 128 * (k + 1), :])
                wt[path, k] = t
        brow_t = const.tile([128, 2 * H], BF16)
        nc.sync.dma_start(out=brow_t[:], in_=brow_d[:])
        eone_t = const.tile([128, 128], BF16)
        nc.sync.dma_start(out=eone_t[:], in_=eone_d[:])
        iota16_t = const.tile([128, 16 * 128], BF16)
        nc.sync.dma_start(out=iota16_t[:], in_=iota16_d[:])
        iota16_3d = iota16_t[:].rearrange("p (a b) -> p a b", b=128)
        tagsA_t = const.tile([128, CA], BF16)
        nc.sync.dma_start(out=tagsA_t[:], in_=tagsA_d[:])
        tagsB_t = const.tile([128, CB], BF16)
        nc.sync.dma_start(out=tagsB_t[:], in_=tagsB_d[:])
        def load_idx(dram, tag):
            t = const.tile([128, dram.shape[1]], I16, tag=tag)
            nc.sync.dma_start(out=t[:], in_=dram[:, :])
            return t

        idx_t = [
            [load_idx(idx_d[q][r], f"idx{q}_{r}") for r in range(NR)]
            for q in range(NQ)
        ]

        nrep = int(os.environ.get("KREPEAT", "1"))  # perf probing only
        qctr = [0]

        def pick_q(q):
            picked = qctr[0] % NQ if KQMODE == "rr" else q
            qctr[0] += 1
            return picked

        for _rep in range(nrep):
            g_tiles = {}

            def emit_gathers(r):
                for q in range(NQ):
                    span = int(
                        seg_start[q][min(NBLK, GG * (r + 1))] - seg_start[q][GG * r]
                    )
                    g = gpools[q].tile([128, SMAX[q], F], BF16, tag=f"g{q}")
                    nc.gpsimd.dma_gather(
                        g[:, 0:span, :],
                        emb16[CW * q :, :],
                        idx_t[q][r][:, :],
                        span * 128,
                        span * 128,
                        F,
                        single_packet=False,
                        queue_num=pick_q(q),
                    )
                    g_tiles[q, r] = g

            # selection matrices, built in 16-column tiles (pool-throttled)
            a_tiles = []
            for u in range(ATILES):
                lo, hi = 16 * u, min(CA, 16 * u + 16)
                at = apool.tile([128, 16, 128], BF16, tag="a")
                nc.vector.tensor_tensor(
                    out=at[:, 0 : hi - lo, :],
                    in0=tagsA_t[:, lo:hi].to_broadcast([128, hi - lo, 128]),
                    in1=iota16_3d[:, 0 : hi - lo, :],
                    op=mybir.AluOpType.is_equal,
                )
                a_tiles.append(at)
            b_tiles = []
            for u in range(BTILES):
                lo, hi = 16 * u, min(CB, 16 * u + 16)
                bt = bpool.tile([128, 16, 128], BF16, tag="b")
                nc.vector.tensor_tensor(
                    out=bt[:, 0 : hi - lo, :],
                    in0=tagsB_t[:, lo:hi].to_broadcast([128, hi - lo, 128]),
                    in1=iota16_3d[:, 0 : hi - lo, :],
                    op=mybir.AluOpType.is_equal,
                )
                b_tiles.append(bt)

            ca = cb = 0
            for blk in range(NBLK):
                if blk % GG == 0:
                    emit_gathers(blk // GG)
                psn = ps_sel.tile([128, 2 * 128], F32, tag="pn")
                pss = ps_sel.tile([128, 2 * 128], F32, tag="ps")
                ncols = int(slots[:, blk].sum())
                done = 0
                for q in range(NQ):
                    r = blk // GG
                    g = g_tiles[q, r]
                    rel = int(seg_start[q][blk] - seg_start[q][GG * r])
                    for s in range(int(slots[q][blk])):
                        acol = a_tiles[ca // 16][:, ca % 16, :]
                        for fh in range(2):
                            # one start/stop per PSUM bank: start marks the
                            # whole 2KB zero region pending-zero, so only the
                            # first matmul into the bank may carry it
                            lhs = g[:, rel + s, 128 * fh : 128 * (fh + 1)]
                            nc.tensor.matmul(
                                out=psn[:, 128 * fh : 128 * (fh + 1)],
                                lhsT=lhs,
                                rhs=acol,
                                start=(done == 0 and fh == 0),
                                stop=(done == ncols - 1 and fh == 1),
                                skip_group_check=True,
                            )
                            if s == 0:
                                nc.tensor.matmul(
                                    out=pss[:, 128 * fh : 128 * (fh + 1)],
                                    lhsT=lhs,
                                    rhs=b_tiles[cb // 16][:, cb % 16, :],
                                    start=(q == 0 and fh == 0),
                                    stop=(q == NQ - 1 and fh == 1),
                                    skip_group_check=True,
                                )
                        ca += 1
                        done += 1
                    cb += 1

                ostage = opool.tile([128, 4 * 128], F32, tag="ostage")
                # group same-func ScalarE ops (Copy x2, later Relu x2) to
                # avoid activation-table reloads between them; ACT is the
                # engine with slack (DVE is loaded with is_equal builds)
                sums = {}
                for path, ps in (("s", pss), ("n", psn)):
                    sumt = sumpool.tile([128, 2 * 128], BF16, tag="sum")
                    nc.scalar.activation(
                        out=sumt[:],
                        in_=ps[:],
                        func=mybir.ActivationFunctionType.Copy,
                    )
                    sums[path] = sumt
                pos = {}
                for pi, path in enumerate(("s", "n")):
                    po = ps_out.tile([128, 2 * 128], F32, tag=f"po{path}")
                    for k in range(2):
                        nc.tensor.matmul(
                            out=po[:],
                            lhsT=sums[path][:, 128 * k : 128 * (k + 1)],
                            rhs=wt[path, k][:],
                            start=(k == 0),
                            stop=False,
                        )
                    nc.tensor.matmul(
                        out=po[:],
                        lhsT=eone_t[:],
                        rhs=brow_t[:, 256 * pi : 256 * (pi + 1)],
                        start=False,
                        stop=True,
                    )
                    pos[path] = po
                for pi, path in enumerate(("s", "n")):
                    nc.scalar.activation(
                        out=ostage[:, 256 * pi : 256 * (pi + 1)],
                        in_=pos[path][:],
                        func=mybir.ActivationFunctionType.Relu,
                    )
                nc.sync.dma_start(
                    out=out[128 * blk : 128 * (blk + 1), :], in_=ostage[:]
                )
            assert ca == CA and cb == CB

    nc.compile()
    return nc


def _prep_and_build(x0, x1):
    structure, per_core = _host_prep(x0, x1)
    key = (
        structure["slots"].tobytes(),
        structure["CA"],
        os.environ.get("KREPEAT", "1"),
        os.environ.get("KGBUFS", "3"),
        GG,
        KQMODE,
    )
    if _BUILT.get("key") != key:
        _BUILT["nc"] = _build(structure)
        _BUILT["key"] = key
    return _BUILT["nc"], structure, per_core


def make_in_maps(x0, x1, emb, W_self, b_self, W_neigh, b_neigh):
    nc, structure, per_core = _prep_and_build(x0, x1)
    emb16 = np.ascontiguousarray(
        np.asarray(emb, dtype=np.float32).astype(ml_dtypes.bfloat16)
    )
    wstv = np.ascontiguousarray(
        np.asarray(W_self, dtype=np.float32).T.astype(ml_dtypes.bfloat16)
    )
    wntv = np.ascontiguousarray(
        (np.asarray(W_neigh, dtype=np.float32).T / NNEIGH).astype(ml_dtypes.bfloat16)
    )
    brow = np.zeros((128, 2 * H), np.float32)
    brow[0, :H] = np.asarray(b_self, dtype=np.float32)
    brow[0, H:] = np.asarray(b_neigh, dtype=np.float32)
    brow = np.ascontiguousarray(brow.astype(ml_dtypes.bfloat16))
    eone = np.zeros((128, 128), np.float32)
    eone[0, :] = 1.0
    eone = np.ascontiguousarray(eone.astype(ml_dtypes.bfloat16))
    iota16 = np.ascontiguousarray(
        np.tile(np.arange(128, dtype=np.float32), (128, 16)).astype(ml_dtypes.bfloat16)
    )
    NR = -(-NBLK // GG)
    in_maps = []
    for c in range(N_CORES):
        m = {
            "emb16": emb16,
            "wst": wstv,
            "wnt": wntv,
            "brow": brow,
            "eone": eone,
            "iota16": iota16,
            "tagsA": per_core[c]["tagsA"],
            "tagsB": per_core[c]["tagsB"],
        }
        for q in range(NQ):
            for r in range(NR):
                m[f"idx{q}_{r}"] = per_core[c]["idx"][q][r]
        in_maps.append(m)
    return nc, in_maps


def kernel(x0, x1, emb, W_self, b_self, W_neigh, b_neigh, **_ignored):
    nc, in_maps = make_in_maps(x0, x1, emb, W_self, b_self, W_neigh, b_neigh)
    res = run_bass_kernel_spmd(nc, in_maps, core_ids=list(range(N_CORES)))
    return np.concatenate([r["out"] for r in res.results], axis=0)
